# revision 39
# baseline (speedup 1.0000x reference)
"""Trainium2 Bass kernel for causal self-attention (B=2, S=2048, D=1024, H=16).

Sharding: 8 cores = 2 batches x 4 head-groups. Core c handles batch c//4 and
heads 4*(c%4) .. 4*(c%4)+4. No cross-core communication; the host gathers the
output slices. w_o is unused by the reference.

Per-core kernel (Tile framework), fp16 projection/scores with fp8 DoubleRow
AV, fp32 psum/softmax:
  1. Inputs DMA upfront: xT chunks on the sync queue, w_qk then w_v on the
     scalar queue, so the DMA-fed first projection drains two chunks at a
     time and attention starts ~14us in.
  2. Projection (fp16, w stationary) produces qT/kT [cols, s]; v is produced
     in natural [s, hd] layout and stored as v||ones (vaug) in fp8e4 (padded
     to 128B chunks for dual-fp8 LDWEIGHTS alignment) plus an fp16 copy of
     j-blocks 0-3 for the fp16 chunk-0 AV.
  3. Scores ST[j,i] = k_j . q_i per j-block, two heads packed into PE rows
     0-63 / 64-127, trimmed to the causal window. Causal masking happens
     inside the same PSUM accumulation group: one extra matmul per head adds
     A^T @ Bwide = -235*(j - i + off) over the entire invalid region
     (A[r,j]=[r<=j] stationary, Bwide[r,c]=-235*[r>c-512] moving), driving
     invalid scores <= -171 so the full-width exp yields exact zeros - no
     other engine touches the ST->exp critical path and no memsets needed.
  4. exp on ACT (bias -3.25, softmax-invariant; true max scaled score ~7.95
     so fp8 p stays well under the 240 e4m3 limit). Chunk t=0 keeps p in
     fp16 with fp16 AV: few-key rows reproduce single v rows where fp8
     quantization would exceed the error budget. Chunks 1-3 write p in fp8
     and run AV in DoubleRow over j-block pairs (K=256/instruction, 0.5
     cyc/col) with v||ones stationary so denominators come for free.
  5. Finalize: PE transpose of outT to natural layout + reciprocal*mul on
     DVE. PSUM: st 2x2 banks + o 2 + pj 1 + fin 1 = 8; fin separate from pj
     so the next chunk's projection never WAR-chains behind finalize.
"""

import sys

sys.path.insert(0, "/opt/trn_rl_repo")

from contextlib import ExitStack

import numpy as np

import concourse.bass as bass
import concourse.tile as tile
from concourse import bacc, masks, mybir
from concourse.bass_utils import run_bass_kernel_spmd

B, S, D, H = 2, 2048, 1024, 16
HD = 64          # head dim
HPC = 4          # heads per core
NCORES = 8
P = 128
NS = S // P      # 16 s-blocks
KC = D // P      # 8 d-chunks
CH = 512         # query-chunk width
NT = S // CH     # 4 query chunks
F32 = mybir.dt.float32
F16 = mybir.dt.float16
F8 = mybir.dt.float8e4
SCALE = 1.0 / np.sqrt(HD)
EXPB = -3.25     # exp(s*SCALE + EXPB): softmax-invariant shift; true max
                 # scaled score is ~7.95, keeps fp8 p max near e^4.7 ~ 110
MASKC = 235.0    # per-step causal mask decrement; one step already zeroes exp
DR = mybir.MatmulPerfMode.DoubleRow

PSUM = bass.MemorySpace.PSUM


def _build_body(ctx: ExitStack, tc: "tile.TileContext", xt_d, wv_d, w16_d, o_d):
    nc = tc.nc

    persist = ctx.enter_context(tc.tile_pool(name="persist", bufs=1))
    ident_h = persist.tile([P, P], F16)
    masks.make_identity(nc, ident_h[:])

    # per-partition bias AP for exp(s*SCALE + EXPB)
    expb = persist.tile([P, 1], F32)
    nc.gpsimd.memset(expb[:], EXPB)

    # Causal-mask matmul operands: A[r,j] = [r<=j] (stationary) and
    # Bwide[r,c] = -MASKC*[r > c-512] (moving); for a diagonal block at
    # column offset `off`, A^T @ Bwide[:, 512-off : 640-off] equals
    # -MASKC*(j - i + off) over window columns [0, off+128): strictly
    # negative multiples of MASKC on the whole invalid region.
    maskA = persist.tile([P, P], F16)
    nc.gpsimd.memset(maskA[:], 1.0)
    nc.gpsimd.affine_select(
        out=maskA[:],
        in_=maskA[:],
        compare_op=mybir.AluOpType.is_ge,
        fill=0.0,
        base=0,
        channel_multiplier=-1,
        pattern=[[1, P]],
    )
    maskB = persist.tile([P, 640], F16)
    nc.gpsimd.memset(maskB[:], -MASKC)
    nc.gpsimd.affine_select(
        out=maskB[:],
        in_=maskB[:],
        compare_op=mybir.AluOpType.is_ge,
        fill=0.0,
        base=511,
        channel_multiplier=1,
        pattern=[[-1, 640]],
    )

    # v in natural layout + ones column, per (j-block, head): fp8 for the
    # DoubleRow AV (chunks 1-3; padded to 128B chunks for dual-fp8 LDWEIGHTS
    # alignment) and fp16 for chunk 0 (j-blocks 0-3 only).
    vaug8 = persist.tile([P, NS, HPC, P], F8)
    nc.gpsimd.memset(vaug8[:, :, :, 64:65], 1.0)
    vaug16 = persist.tile([P, 4 * HPC * 65], F16)
    nc.vector.memset(
        vaug16[:].rearrange("p (n c) -> p n c", c=65)[:, :, 64:65], 1.0
    )
    out_sb = persist.tile([P, NS * HPC * HD], F32)

    # ---- static SBUF inputs -------------------------------------------
    wp = ctx.enter_context(tc.tile_pool(name="w", bufs=1))
    xT = wp.tile([P, KC * S], F16)             # 32KB/part, all chunks
    wv_sb = wp.tile([P, KC * 256], F16)        # 4KB/part
    w16_sb = wp.tile([P, KC * 4 * P], F16)     # 8KB/part, q/k weights

    # Batched strided DMAs (each DMA instruction occupies its queue for the
    # whole transfer and carries ~0.5us of semaphore overhead, so fewer and
    # bigger is faster): xT on sync, w_qk on scalar (drained before the
    # first exp needs the queue), w_v on gpsimd (idle after the memsets).
    w16_v = w16_d.rearrange("(k p) c -> p k c", p=P)
    w16s_v = w16_sb[:].rearrange("p (k c) -> p k c", k=KC)
    wv_v = wv_d.rearrange("(k p) c -> p k c", p=P)
    wvs_v = wv_sb[:].rearrange("p (k c) -> p k c", k=KC)
    for kh in range(2):
        k4 = slice(4 * kh, 4 * kh + 4)
        nc.scalar.dma_start(w16s_v[:, k4, :], w16_v[:, k4, :])
    nc.gpsimd.dma_start(wvs_v[:], wv_v[:])
    # xT per (k-chunk, S-half): contiguous destination ranges with disjoint
    # bounding boxes, so chunk-t reads never pick up false dependencies on
    # later-chunk DMAs (range tracking is bounding-box based).
    for half in range(2):
        for k in range(KC):
            c0 = half * 2 * CH
            nc.sync.dma_start(
                xT[:, k * S + c0: k * S + c0 + 2 * CH],
                xt_d[k * P:(k + 1) * P, c0:c0 + 2 * CH],
            )

    # ---- pools ---------------------------------------------------------
    qkp = ctx.enter_context(tc.tile_pool(name="qk", bufs=1))
    qkT = qkp.tile([P, 4 * S], F16)      # m0,m1 = q(h01,h23); m2,m3 = k

    ps_st = ctx.enter_context(tc.tile_pool(name="ps_st", bufs=2, space=PSUM))
    ps_o = ctx.enter_context(tc.tile_pool(name="ps_o", bufs=2, space=PSUM))
    ps_pj = ctx.enter_context(tc.tile_pool(name="ps_pj", bufs=1, space=PSUM))
    ps_fin = ctx.enter_context(tc.tile_pool(name="ps_fin", bufs=1, space=PSUM))
    pp8 = ctx.enter_context(tc.tile_pool(name="p8", bufs=6))
    pp16 = ctx.enter_context(tc.tile_pool(name="p16", bufs=6))
    osbp = ctx.enter_context(tc.tile_pool(name="osb", bufs=4))
    rcp = ctx.enter_context(tc.tile_pool(name="rcol", bufs=4))

    def projqk(m, t):
        """qkT[:, m*S + t-chunk] = w_qk[col-block m]^T @ x^T."""
        pj = ps_pj.tile([P, CH], F32, tag="pj")
        for k in range(KC):
            nc.tensor.matmul(
                pj[:],
                w16_sb[:, k * 4 * P + m * P: k * 4 * P + (m + 1) * P],
                xT[:, k * S + t * CH: k * S + (t + 1) * CH],
                start=(k == 0),
                stop=(k == KC - 1),
            )
        nc.vector.tensor_copy(
            qkT[:, m * S + t * CH: m * S + (t + 1) * CH], pj[:]
        )

    def vdirect(sb):
        """vaug[s-block sb] = x[sb] @ w_v (natural layout), all 4 heads."""
        pv = ps_pj.tile([P, CH], F32, tag="pj")
        for k in range(KC):
            nc.tensor.matmul(
                pv[:, 0:256],
                xT[:, k * S + sb * P: k * S + (sb + 1) * P],
                wv_sb[:, k * 256:(k + 1) * 256],
                start=(k == 0),
                stop=(k == KC - 1),
            )
        nc.vector.tensor_copy(
            vaug8[:, sb, :, 0:64],
            pv[:, 0:256].rearrange("p (g c) -> p g c", c=64),
        )
        if sb < 4:
            nc.vector.tensor_copy(
                vaug16[:, sb * HPC * 65:(sb + 1) * HPC * 65]
                .rearrange("p (g c) -> p g c", c=65)[:, :, 0:64],
                pv[:, 0:256].rearrange("p (g c) -> p g c", c=64),
            )

    out_view = out_sb[:].rearrange("p (i g d) -> p i g d", g=HPC, d=HD)

    def finalize(h, t, po_h):
        """Transpose outT to natural layout, divide by denominator."""
        osb = osbp.tile([65, CH], F16, tag="osb")
        nc.vector.tensor_copy(osb[:], po_h[:])
        fin32 = ps_fin.tile([P, CH], F32, tag="fin")
        fin = fin32.bitcast(F16)[:, 0:CH]
        for b4 in range(4):
            nc.tensor.transpose(
                fin[:, b4 * P:b4 * P + 65],
                osb[:, b4 * P:(b4 + 1) * P],
                ident_h[0:65, 0:65],
            )
        fin_view = fin[:, 0:CH].rearrange("p (n c) -> p n c", c=P)
        rc = rcp.tile([P, 4], F32, tag="rc")
        nc.vector.reciprocal(rc[:], fin_view[:, :, 64])
        nc.vector.tensor_mul(
            out_view[:, 4 * t:4 * t + 4, h, :],
            fin_view[:, :, 0:64],
            rc[:].broadcast_to([P, 4, HD]),
        )

    def st_block(pair, t, jb, hA, hB):
        """Scores^T for one j-block, both heads, causal-masked in-group."""
        qm, km = pair, 2 + pair
        doff = jb - 4 * t
        off = P * doff if doff > 0 else 0
        diag = doff >= 0
        st = ps_st.tile([P, 1024], F32, tag="st")
        for hi, h in enumerate((hA, hB)):
            hb = (h % 2) * 64
            nc.tensor.matmul(
                st[:, hi * CH + off:(hi + 1) * CH],
                qkT[hb:hb + 64, km * S + jb * P: km * S + (jb + 1) * P],
                qkT[hb:hb + 64, qm * S + t * CH + off: qm * S + (t + 1) * CH],
                start=True,
                stop=not diag,
                tile_position=(hb, 0),
            )
            if diag:
                # triangular mask accumulated onto the diagonal block
                nc.tensor.matmul(
                    st[:, hi * CH + off: hi * CH + off + P],
                    maskA[:],
                    maskB[:, CH:CH + P],
                    start=False,
                    stop=True,
                )
                if off:
                    # below-window columns: overwrite with -MASKC*(j+1)
                    nc.tensor.matmul(
                        st[:, hi * CH: hi * CH + off],
                        maskA[:],
                        maskB[:, 0:off],
                        start=True,
                        stop=True,
                    )
        return st

    def attn_t0(pair):
        """Query chunk 0 (rows 0-511): fp16 p and fp16 AV (few-key rows)."""
        hA, hB = 2 * pair, 2 * pair + 1
        po = {hA: ps_o.tile([65, CH], F32, tag="o", name="po_a"),
              hB: ps_o.tile([65, CH], F32, tag="o", name="po_b")}
        for jb in range(4):
            st = st_block(pair, 0, jb, hA, hB)
            off = P * jb
            p16 = pp16.tile([P, 1024], F16, tag="p16")
            nc.scalar.activation(
                p16[:], st[:], mybir.ActivationFunctionType.Exp,
                scale=float(SCALE), bias=expb[:],
            )
            for hi, h in enumerate((hA, hB)):
                nc.tensor.matmul(
                    po[h][:, off:CH],
                    vaug16[:, (jb * HPC + h) * 65:(jb * HPC + h + 1) * 65],
                    p16[:, hi * CH + off:(hi + 1) * CH],
                    start=(jb == 0),
                    stop=(jb == 3),
                )
        for h in (hA, hB):
            finalize(h, 0, po[h])

    def attn(pair, t):
        """Query chunk t>=1: fp8 p, DoubleRow AV over j-block pairs."""
        hA, hB = 2 * pair, 2 * pair + 1
        njb = 4 * t + 4
        npair = njb // 2
        po = {hA: ps_o.tile([65, CH], F32, tag="o", name="po_a"),
              hB: ps_o.tile([65, CH], F32, tag="o", name="po_b")}
        for jp in range(npair):
            p8 = pp8.tile([P, 2, 1024], F8, tag="p8")
            for half in (0, 1):
                jb = 2 * jp + half
                st = st_block(pair, t, jb, hA, hB)
                nc.scalar.activation(
                    p8[:, half, :], st[:], mybir.ActivationFunctionType.Exp,
                    scale=float(SCALE), bias=expb[:],
                )
            for hi, h in enumerate((hA, hB)):
                nc.tensor.matmul(
                    po[h][:],
                    vaug8[:, 2 * jp:2 * jp + 2, h, 0:65],
                    p8[:, :, hi * CH:(hi + 1) * CH],
                    start=(jp == 0),
                    stop=(jp == npair - 1),
                    perf_mode=DR,
                )
        for h in (hA, hB):
            finalize(h, t, po[h])

    # ---- main loop -----------------------------------------------------
    # Emission order = PE priority. Chunk t's pair-0 attention comes right
    # after its projections; chunk t+1's projections are emitted between the
    # two attention pairs so the PE fills ACT-bound stretches with them.
    def proj_chunk(t):
        for m in (0, 2, 1, 3):
            projqk(m, t)
        for sb in range(4 * t, 4 * t + 4):
            vdirect(sb)

    def out_dma(t):
        for b4 in range(4):
            ib = 4 * t + b4
            nc.sync.dma_start(
                o_d[ib * P:(ib + 1) * P, :],
                out_sb[:, ib * HPC * HD:(ib + 1) * HPC * HD],
            )

    projqk(0, 0)
    projqk(2, 0)
    for sb in range(4):
        vdirect(sb)
    attn_t0(0)
    projqk(1, 0)
    projqk(3, 0)
    proj_chunk(1)
    attn_t0(1)
    out_dma(0)
    for t in range(1, NT):
        attn(0, t)
        if t < NT - 1:
            proj_chunk(t + 1)
        attn(1, t)
        out_dma(t)


def build_program():
    nc = bacc.Bacc(
        "TRN2",
        target_bir_lowering=False,
        debug=False,
        enable_asserts=True,
    )
    xt_d = nc.dram_tensor("xT", [D, S], F16, kind="ExternalInput").ap()
    wv_d = nc.dram_tensor("wv", [D, 256], F16, kind="ExternalInput").ap()
    w16_d = nc.dram_tensor("wqk16", [D, 4 * P], F16, kind="ExternalInput").ap()
    o_d = nc.dram_tensor("o", [S, HPC * HD], F32, kind="ExternalOutput").ap()

    with tile.TileContext(nc) as tc, ExitStack() as ctx:
        _build_body(ctx, tc, xt_d, wv_d, w16_d, o_d)
    nc.compile()
    return nc


_CACHE = {}


def _compiled():
    if "nc" not in _CACHE:
        _CACHE["nc"] = build_program()
    return _CACHE["nc"]


def make_in_maps(x, w_qkv):
    x = np.asarray(x, dtype=np.float32)
    w_qkv = np.asarray(w_qkv, dtype=np.float32)
    xT16 = [np.ascontiguousarray(x[b].T).astype(np.float16) for b in range(B)]
    in_maps = []
    for c in range(NCORES):
        b = c // 4
        cs = (c % 4) * HPC * HD
        wqk = np.concatenate(
            [w_qkv[:, cs:cs + 256], w_qkv[:, D + cs:D + cs + 256]], axis=1
        )
        wv = np.ascontiguousarray(w_qkv[:, 2 * D + cs:2 * D + cs + 256]).astype(
            np.float16
        )
        in_maps.append(
            {"xT": xT16[b], "wv": wv, "wqk16": wqk.astype(np.float16)}
        )
    return in_maps


def gather_out(results):
    out = np.empty((B, S, D), np.float32)
    for c in range(NCORES):
        b = c // 4
        cs = (c % 4) * HPC * HD
        out[b][:, cs:cs + HPC * HD] = results[c]["o"]
    return out


def kernel(x, w_qkv, w_o=None, **_):
    nc = _compiled()
    res = run_bass_kernel_spmd(nc, make_in_maps(x, w_qkv), core_ids=list(range(NCORES)))
    return gather_out(res.results)


# revision 43
# speedup vs baseline: 1.0348x; 1.0348x over previous
"""Trainium2 Bass kernel for causal self-attention (B=2, S=2048, D=1024, H=16).

Sharding: 8 cores = 2 batches x 4 head-groups. Core c handles batch c//4 and
heads 4*(c%4) .. 4*(c%4)+4. No cross-core communication; the host gathers the
output slices. w_o is unused by the reference.

Per-core kernel (Tile framework), fp16 projection/scores with fp8 DoubleRow
AV, fp32 psum/softmax:
  1. Inputs DMA upfront: xT chunks on the sync queue, w_qk then w_v on the
     scalar queue, so the DMA-fed first projection drains two chunks at a
     time and attention starts ~14us in.
  2. Projection (fp16, w stationary) produces qT/kT [cols, s]; v is produced
     in natural [s, hd] layout and stored as v||ones (vaug) in fp8e4 (padded
     to 128B chunks for dual-fp8 LDWEIGHTS alignment) plus an fp16 copy of
     j-blocks 0-3 for the fp16 chunk-0 AV.
  3. Scores ST[j,i] = k_j . q_i per j-block, two heads packed into PE rows
     0-63 / 64-127, trimmed to the causal window. Causal masking happens
     inside the same PSUM accumulation group: one extra matmul per head adds
     A^T @ Bwide = -235*(j - i + off) over the entire invalid region
     (A[r,j]=[r<=j] stationary, Bwide[r,c]=-235*[r>c-512] moving), driving
     invalid scores <= -171 so the full-width exp yields exact zeros - no
     other engine touches the ST->exp critical path and no memsets needed.
  4. exp on ACT (bias -3.25, softmax-invariant; true max scaled score ~7.95
     so fp8 p stays well under the 240 e4m3 limit). Chunk t=0 keeps p in
     fp16 with fp16 AV: few-key rows reproduce single v rows where fp8
     quantization would exceed the error budget. Chunks 1-3 write p in fp8
     and run AV in DoubleRow over j-block pairs (K=256/instruction, 0.5
     cyc/col) with v||ones stationary so denominators come for free.
  5. Finalize: PE transpose of outT to natural layout + reciprocal*mul on
     DVE. PSUM: st 2x2 banks + o 2 + pj 1 + fin 1 = 8; fin separate from pj
     so the next chunk's projection never WAR-chains behind finalize.
"""

import sys

sys.path.insert(0, "/opt/trn_rl_repo")

from contextlib import ExitStack

import numpy as np

import concourse.bass as bass
import concourse.tile as tile
from concourse import bacc, masks, mybir
from concourse.bass_utils import run_bass_kernel_spmd

B, S, D, H = 2, 2048, 1024, 16
HD = 64          # head dim
HPC = 4          # heads per core
NCORES = 8
P = 128
NS = S // P      # 16 s-blocks
KC = D // P      # 8 d-chunks
CH = 512         # query-chunk width
NT = S // CH     # 4 query chunks
F32 = mybir.dt.float32
F16 = mybir.dt.float16
F8 = mybir.dt.float8e4
SCALE = 1.0 / np.sqrt(HD)
EXPB = -3.25     # exp(s*SCALE + EXPB): softmax-invariant shift; true max
                 # scaled score is ~7.95, keeps fp8 p max near e^4.7 ~ 110
MASKC = 235.0    # per-step causal mask decrement; one step already zeroes exp
DR = mybir.MatmulPerfMode.DoubleRow

PSUM = bass.MemorySpace.PSUM


def _build_body(ctx: ExitStack, tc: "tile.TileContext", xt_d, wv_d, w16_d, o_d):
    nc = tc.nc

    persist = ctx.enter_context(tc.tile_pool(name="persist", bufs=1))
    ident_h = persist.tile([P, P], F16)
    masks.make_identity(nc, ident_h[:])

    # per-partition bias AP for exp(s*SCALE + EXPB)
    expb = persist.tile([P, 1], F32)
    nc.gpsimd.memset(expb[:], EXPB)

    # Causal-mask matmul operands: A[r,j] = [r<=j] (stationary) and
    # Bwide[r,c] = -MASKC*[r > c-512] (moving); for a diagonal block at
    # column offset `off`, A^T @ Bwide[:, 512-off : 640-off] equals
    # -MASKC*(j - i + off) over window columns [0, off+128): strictly
    # negative multiples of MASKC on the whole invalid region.
    maskA = persist.tile([P, P], F16)
    nc.gpsimd.memset(maskA[:], 1.0)
    nc.gpsimd.affine_select(
        out=maskA[:],
        in_=maskA[:],
        compare_op=mybir.AluOpType.is_ge,
        fill=0.0,
        base=0,
        channel_multiplier=-1,
        pattern=[[1, P]],
    )
    maskB = persist.tile([P, 640], F16)
    nc.gpsimd.memset(maskB[:], -MASKC)
    nc.gpsimd.affine_select(
        out=maskB[:],
        in_=maskB[:],
        compare_op=mybir.AluOpType.is_ge,
        fill=0.0,
        base=511,
        channel_multiplier=1,
        pattern=[[-1, 640]],
    )

    # v in natural layout + ones column, per (j-block, head): fp8 for the
    # DoubleRow AV (chunks 1-3; padded to 128B chunks for dual-fp8 LDWEIGHTS
    # alignment) and fp16 for chunk 0 (j-blocks 0-3 only).
    vaug8 = persist.tile([P, NS, HPC, P], F8)
    nc.gpsimd.memset(vaug8[:, :, :, 64:65], 1.0)
    vaug16 = persist.tile([P, 4 * HPC * 65], F16)
    nc.vector.memset(
        vaug16[:].rearrange("p (n c) -> p n c", c=65)[:, :, 64:65], 1.0
    )
    out_sb = persist.tile([P, NS * HPC * HD], F32)

    # ---- static SBUF inputs -------------------------------------------
    wp = ctx.enter_context(tc.tile_pool(name="w", bufs=1))
    xT = wp.tile([P, KC * S], F16)             # 32KB/part, all chunks
    wv_sb = wp.tile([P, KC * 256], F16)        # 4KB/part
    w16_sb = wp.tile([P, KC * 4 * P], F16)     # 8KB/part, q/k weights

    # Batched strided DMAs (each DMA instruction occupies its queue for the
    # whole transfer and carries ~0.5us of semaphore overhead, so fewer and
    # bigger is faster): xT on sync, w_qk on scalar (drained before the
    # first exp needs the queue), w_v on gpsimd (idle after the memsets).
    w16_v = w16_d.rearrange("(k p) c -> p k c", p=P)
    w16s_v = w16_sb[:].rearrange("p (k c) -> p k c", k=KC)
    wv_v = wv_d.rearrange("(k p) c -> p k c", p=P)
    wvs_v = wv_sb[:].rearrange("p (k c) -> p k c", k=KC)
    for kh in range(2):
        k4 = slice(4 * kh, 4 * kh + 4)
        nc.scalar.dma_start(w16s_v[:, k4, :], w16_v[:, k4, :])
    nc.gpsimd.dma_start(wvs_v[:], wv_v[:])
    # xT per (k-chunk, S-half): contiguous destination ranges with disjoint
    # bounding boxes, so chunk-t reads never pick up false dependencies on
    # later-chunk DMAs (range tracking is bounding-box based).
    for half in range(2):
        for k in range(KC):
            c0 = half * 2 * CH
            nc.sync.dma_start(
                xT[:, k * S + c0: k * S + c0 + 2 * CH],
                xt_d[k * P:(k + 1) * P, c0:c0 + 2 * CH],
            )

    # ---- pools ---------------------------------------------------------
    qkp = ctx.enter_context(tc.tile_pool(name="qk", bufs=1))
    qkT = qkp.tile([P, 4 * S], F16)      # m0,m1 = q(h01,h23); m2,m3 = k

    ps_st = ctx.enter_context(tc.tile_pool(name="ps_st", bufs=2, space=PSUM))
    ps_o = ctx.enter_context(tc.tile_pool(name="ps_o", bufs=2, space=PSUM))
    ps_pj = ctx.enter_context(tc.tile_pool(name="ps_pj", bufs=1, space=PSUM))
    ps_fin = ctx.enter_context(tc.tile_pool(name="ps_fin", bufs=1, space=PSUM))
    pp8 = ctx.enter_context(tc.tile_pool(name="p8", bufs=6))
    pp16 = ctx.enter_context(tc.tile_pool(name="p16", bufs=6))
    osbp = ctx.enter_context(tc.tile_pool(name="osb", bufs=4))
    rcp = ctx.enter_context(tc.tile_pool(name="rcol", bufs=4))

    def projqk(m, t):
        """qkT[:, m*S + t-chunk] = w_qk[col-block m]^T @ x^T."""
        pj = ps_pj.tile([P, CH], F32, tag="pj")
        for k in range(KC):
            nc.tensor.matmul(
                pj[:],
                w16_sb[:, k * 4 * P + m * P: k * 4 * P + (m + 1) * P],
                xT[:, k * S + t * CH: k * S + (t + 1) * CH],
                start=(k == 0),
                stop=(k == KC - 1),
            )
        nc.vector.tensor_copy(
            qkT[:, m * S + t * CH: m * S + (t + 1) * CH], pj[:]
        )

    def projqk0_pair(mA, mB):
        """Two t=0 projection chains k-interleaved on two psum banks (fin is
        idle at startup), so each chunk-k DMA arrival feeds two matmuls."""
        pjA = ps_pj.tile([P, CH], F32, tag="pj", name="pjA")
        pjB = ps_fin.tile([P, CH], F32, tag="fin", name="pjB")
        for k in range(KC):
            for m, pj in ((mA, pjA), (mB, pjB)):
                nc.tensor.matmul(
                    pj[:],
                    w16_sb[:, k * 4 * P + m * P: k * 4 * P + (m + 1) * P],
                    xT[:, k * S: k * S + CH],
                    start=(k == 0),
                    stop=(k == KC - 1),
                )
        for m, pj in ((mA, pjA), (mB, pjB)):
            nc.vector.tensor_copy(qkT[:, m * S: m * S + CH], pj[:])

    def vdirect(sb):
        """vaug[s-block sb] = x[sb] @ w_v (natural layout), all 4 heads."""
        pv = ps_pj.tile([P, CH], F32, tag="pj")
        for k in range(KC):
            nc.tensor.matmul(
                pv[:, 0:256],
                xT[:, k * S + sb * P: k * S + (sb + 1) * P],
                wv_sb[:, k * 256:(k + 1) * 256],
                start=(k == 0),
                stop=(k == KC - 1),
            )
        nc.vector.tensor_copy(
            vaug8[:, sb, :, 0:64],
            pv[:, 0:256].rearrange("p (g c) -> p g c", c=64),
        )
        if sb < 4:
            nc.vector.tensor_copy(
                vaug16[:, sb * HPC * 65:(sb + 1) * HPC * 65]
                .rearrange("p (g c) -> p g c", c=65)[:, :, 0:64],
                pv[:, 0:256].rearrange("p (g c) -> p g c", c=64),
            )

    out_view = out_sb[:].rearrange("p (i g d) -> p i g d", g=HPC, d=HD)

    def finalize(h, t, po_h):
        """Transpose outT to natural layout, divide by denominator."""
        osb = osbp.tile([65, CH], F16, tag="osb")
        nc.vector.tensor_copy(osb[:], po_h[:])
        fin32 = ps_fin.tile([P, CH], F32, tag="fin")
        fin = fin32.bitcast(F16)[:, 0:CH]
        for b4 in range(4):
            nc.tensor.transpose(
                fin[:, b4 * P:b4 * P + 65],
                osb[:, b4 * P:(b4 + 1) * P],
                ident_h[0:65, 0:65],
            )
        fin_view = fin[:, 0:CH].rearrange("p (n c) -> p n c", c=P)
        rc = rcp.tile([P, 4], F32, tag="rc")
        nc.vector.reciprocal(rc[:], fin_view[:, :, 64])
        nc.vector.tensor_mul(
            out_view[:, 4 * t:4 * t + 4, h, :],
            fin_view[:, :, 0:64],
            rc[:].broadcast_to([P, 4, HD]),
        )

    def st_block(pair, t, jb, hA, hB):
        """Scores^T for one j-block, both heads, causal-masked in-group."""
        qm, km = pair, 2 + pair
        doff = jb - 4 * t
        off = P * doff if doff > 0 else 0
        diag = doff >= 0
        st = ps_st.tile([P, 1024], F32, tag="st")
        for hi, h in enumerate((hA, hB)):
            hb = (h % 2) * 64
            nc.tensor.matmul(
                st[:, hi * CH + off:(hi + 1) * CH],
                qkT[hb:hb + 64, km * S + jb * P: km * S + (jb + 1) * P],
                qkT[hb:hb + 64, qm * S + t * CH + off: qm * S + (t + 1) * CH],
                start=True,
                stop=not diag,
                tile_position=(hb, 0),
            )
            if diag:
                # triangular mask accumulated onto the diagonal block
                nc.tensor.matmul(
                    st[:, hi * CH + off: hi * CH + off + P],
                    maskA[:],
                    maskB[:, CH:CH + P],
                    start=False,
                    stop=True,
                )
                if off:
                    # below-window columns: overwrite with -MASKC*(j+1)
                    nc.tensor.matmul(
                        st[:, hi * CH: hi * CH + off],
                        maskA[:],
                        maskB[:, 0:off],
                        start=True,
                        stop=True,
                    )
        return st

    def attn_t0(pair, vd_base=None):
        """Query chunk 0 (rows 0-511): fp16 p and fp16 AV (few-key rows)."""
        hA, hB = 2 * pair, 2 * pair + 1
        po = {hA: ps_o.tile([65, CH], F32, tag="o", name="po_a"),
              hB: ps_o.tile([65, CH], F32, tag="o", name="po_b")}
        for jb in range(4):
            if vd_base is not None:
                vdirect(vd_base + jb)
            st = st_block(pair, 0, jb, hA, hB)
            off = P * jb
            p16 = pp16.tile([P, 1024], F16, tag="p16")
            nc.scalar.activation(
                p16[:], st[:], mybir.ActivationFunctionType.Exp,
                scale=float(SCALE), bias=expb[:],
            )
            for hi, h in enumerate((hA, hB)):
                nc.tensor.matmul(
                    po[h][:, off:CH],
                    vaug16[:, (jb * HPC + h) * 65:(jb * HPC + h + 1) * 65],
                    p16[:, hi * CH + off:(hi + 1) * CH],
                    start=(jb == 0),
                    stop=(jb == 3),
                )
        for h in (hA, hB):
            finalize(h, 0, po[h])

    def attn(pair, t, vd_base=None):
        """Query chunk t>=1: fp8 p, DoubleRow AV over j-block pairs."""
        hA, hB = 2 * pair, 2 * pair + 1
        njb = 4 * t + 4
        npair = njb // 2
        po = {hA: ps_o.tile([65, CH], F32, tag="o", name="po_a"),
              hB: ps_o.tile([65, CH], F32, tag="o", name="po_b")}
        for jp in range(npair):
            if vd_base is not None and jp < 4:
                vdirect(vd_base + jp)
            p8 = pp8.tile([P, 2, 1024], F8, tag="p8")
            for half in (0, 1):
                jb = 2 * jp + half
                st = st_block(pair, t, jb, hA, hB)
                nc.scalar.activation(
                    p8[:, half, :], st[:], mybir.ActivationFunctionType.Exp,
                    scale=float(SCALE), bias=expb[:],
                )
            for hi, h in enumerate((hA, hB)):
                nc.tensor.matmul(
                    po[h][:],
                    vaug8[:, 2 * jp:2 * jp + 2, h, 0:65],
                    p8[:, :, hi * CH:(hi + 1) * CH],
                    start=(jp == 0),
                    stop=(jp == npair - 1),
                    perf_mode=DR,
                )
        for h in (hA, hB):
            finalize(h, t, po[h])

    # ---- main loop -----------------------------------------------------
    # Emission order = PE priority among ready instructions: ACT-feeding
    # score streams first, v-projections interleaved per j-block inside the
    # attention calls, and the next chunk's q/k projections last so they
    # soak up PE slack during the ACT-bound attention stretches.
    def out_dma(t):
        for b4 in range(4):
            ib = 4 * t + b4
            nc.sync.dma_start(
                o_d[ib * P:(ib + 1) * P, :],
                out_sb[:, ib * HPC * HD:(ib + 1) * HPC * HD],
            )

    projqk0_pair(0, 2)
    attn_t0(0, vd_base=0)
    projqk0_pair(1, 3)
    attn_t0(1, vd_base=4)      # v for chunk 1
    for m in (0, 2, 1, 3):
        projqk(m, 1)
    out_dma(0)
    for t in range(1, NT):
        attn(0, t)
        attn(1, t, vd_base=(4 * t + 4 if t < NT - 1 else None))
        if t < NT - 1:
            for m in (0, 2, 1, 3):
                projqk(m, t + 1)
        out_dma(t)


def build_program():
    nc = bacc.Bacc(
        "TRN2",
        target_bir_lowering=False,
        debug=False,
        enable_asserts=True,
    )
    xt_d = nc.dram_tensor("xT", [D, S], F16, kind="ExternalInput").ap()
    wv_d = nc.dram_tensor("wv", [D, 256], F16, kind="ExternalInput").ap()
    w16_d = nc.dram_tensor("wqk16", [D, 4 * P], F16, kind="ExternalInput").ap()
    o_d = nc.dram_tensor("o", [S, HPC * HD], F32, kind="ExternalOutput").ap()

    with tile.TileContext(nc) as tc, ExitStack() as ctx:
        _build_body(ctx, tc, xt_d, wv_d, w16_d, o_d)
    nc.compile()
    return nc


_CACHE = {}


def _compiled():
    if "nc" not in _CACHE:
        _CACHE["nc"] = build_program()
    return _CACHE["nc"]


def make_in_maps(x, w_qkv):
    x = np.asarray(x, dtype=np.float32)
    w_qkv = np.asarray(w_qkv, dtype=np.float32)
    xT16 = [np.ascontiguousarray(x[b].T).astype(np.float16) for b in range(B)]
    in_maps = []
    for c in range(NCORES):
        b = c // 4
        cs = (c % 4) * HPC * HD
        wqk = np.concatenate(
            [w_qkv[:, cs:cs + 256], w_qkv[:, D + cs:D + cs + 256]], axis=1
        )
        wv = np.ascontiguousarray(w_qkv[:, 2 * D + cs:2 * D + cs + 256]).astype(
            np.float16
        )
        in_maps.append(
            {"xT": xT16[b], "wv": wv, "wqk16": wqk.astype(np.float16)}
        )
    return in_maps


def gather_out(results):
    out = np.empty((B, S, D), np.float32)
    for c in range(NCORES):
        b = c // 4
        cs = (c % 4) * HPC * HD
        out[b][:, cs:cs + HPC * HD] = results[c]["o"]
    return out


def kernel(x, w_qkv, w_o=None, **_):
    nc = _compiled()
    res = run_bass_kernel_spmd(nc, make_in_maps(x, w_qkv), core_ids=list(range(NCORES)))
    return gather_out(res.results)
